# revision 39
# baseline (speedup 1.0000x reference)
"""Trainium2 Bass kernel for nn_AttentionForQuantizer.

Computes, for hidden_states [32768, 256] and a 4096-entry codebook:
    q = rmsnorm(hs @ wq + bq) * gq ;  k = rmsnorm(cb @ wk + bk) * gk
    logits = (q @ k.T) / sqrt(A)
    idx    = argmax(logits, axis=1)
    z_q = z_q_2 = v[idx]   where v = cb @ wv + bv
(the softmax in the reference cancels out of every returned tensor up to
~1 ulp, so it is never computed).

Sharding: data-parallel over the token axis across 8 NeuronCores; codebook
and weights replicated.  Math identity used on device:
    logits[t, n] = fq[t] * sum_d q_raw[t, d] * gg[d] * rk[n] * k_raw[n, d]
with fq[t] = scale * rsqrt(mean_d q_raw^2 + eps), rk[n] = rsqrt(mean + eps),
gg = gq * gk.  fq folds into the lhsT operand, gg/rk into the rhs operand.

All accuracy-critical matmuls run as fp16 hi/lo pairs with fp32 PSUM
accumulation (3 passes: h0k0 + h0k1 + h1k0; the dropped h1k1 term is
O(2^-22)).  Measured deviation vs float64 is ~3.5e-6 absolute on the logits
-- below plain fp32 matmul reordering noise and 10x below the dataset's
minimum top1-top2 logit gap, so argmax is stable.  Power-of-2 scales keep
the lo parts out of fp16-subnormal range; descales fold into the
(mandatory) PSUM->SBUF eviction ops for free.
"""

import os
import numpy as np

P = 128
T = 4096          # tokens per core
NCODES = 4096     # codebook entries
H = 256           # hidden dim (2 partition tiles)
A = 256           # attn dim   (2 partition tiles)
C = 256           # v output dim
NCH = 512         # n-chunk width (1 PSUM bank of fp32)
NCHUNKS = NCODES // NCH
TT = T // P       # token tiles per core
EPS = 1e-5
N_CORES = 8

SIN = 64.0        # input (hs/cb) scale for fp16 pairs
SW = 128.0        # weight scale
DPROJ = 1.0 / (SIN * SW)      # projection descale 2^-13
SQ = 128.0        # extra scale folded into fq
SK = 32.0         # extra scale folded into rk
DBIG = 1.0 / (SQ * SK)        # big-matmul descale 2^-12


def _build_module(reps=1):
    import concourse.bass as bass
    from concourse import bacc, mybir
    from concourse.tile import TileContext
    from contextlib import ExitStack

    f32 = mybir.dt.float32
    f16 = mybir.dt.float16
    i32 = mybir.dt.int32
    Copy = mybir.ActivationFunctionType.Copy
    Ident = mybir.ActivationFunctionType.Identity
    Sqrt = mybir.ActivationFunctionType.Sqrt
    Square = mybir.ActivationFunctionType.Square
    alu = mybir.AluOpType
    AX = mybir.AxisListType.X

    nc = bacc.Bacc()

    # ---- I/O ----
    hsT_d = nc.declare_dram_parameter("hsT", [H, T], f32, isOutput=False)
    cbT_d = nc.declare_dram_parameter("cbT", [H, NCODES], f32, isOutput=False)
    wq_d = nc.declare_dram_parameter("wq", [H, A], f32, isOutput=False)
    wk_d = nc.declare_dram_parameter("wk", [H, A], f32, isOutput=False)
    wv_d = nc.declare_dram_parameter("wv", [H, C], f32, isOutput=False)
    bq_d = nc.declare_dram_parameter("bq", [1, A], f32, isOutput=False)
    bk_d = nc.declare_dram_parameter("bk", [1, A], f32, isOutput=False)
    bv_d = nc.declare_dram_parameter("bv", [1, C], f32, isOutput=False)
    gq_d = nc.declare_dram_parameter("gq", [1, A], f32, isOutput=False)
    gk_d = nc.declare_dram_parameter("gk", [1, A], f32, isOutput=False)
    iota_d = nc.declare_dram_parameter("iota", [1, NCODES], f32, isOutput=False)

    logits_o = nc.declare_dram_parameter("logits", [T, NCODES], f32, isOutput=True)
    idx_o = nc.declare_dram_parameter("idx", [T, 1], i32, isOutput=True)
    zq_o = nc.declare_dram_parameter("zq", [T, C], f32, isOutput=True)
    zq2_o = nc.declare_dram_parameter("zq2", [T, C], f32, isOutput=True)

    hsT_r = hsT_d[:].rearrange("(a p) t -> p a t", p=P)
    cbT_r = cbT_d[:].rearrange("(a p) n -> p a n", p=P)

    with TileContext(nc) as tc, ExitStack() as ctx:
        pool = lambda name, bufs, space="SBUF": ctx.enter_context(
            tc.tile_pool(name=name, bufs=bufs, space=space)
        )

        const = pool("const", 1)
        dram = pool("dram", 1, "DRAM")
        ps_big = pool("ps_big", 4, "PSUM")
        ps_v = pool("ps_v", 2, "PSUM")
        ps_row = pool("ps_row", 2, "PSUM")

        # ---- weights: scaled fp16 hi/lo pairs (scratch released after) ----
        wsb = {}
        with tc.tile_pool(name="wtmp", bufs=2) as wtmp:
            for name, wd in (("wq", wq_d), ("wk", wk_d), ("wv", wv_d)):
                w32 = wtmp.tile([P, 2, A], f32, tag="w32")
                nc.sync.dma_start(out=w32,
                                  in_=wd[:].rearrange("(a p) d -> p a d", p=P))
                h0 = const.tile([P, 2, A], f16, tag=f"{name}h0")
                h1 = const.tile([P, 2, A], f16, tag=f"{name}h1")
                nc.vector.tensor_scalar(out=h0, in0=w32, scalar1=SW, scalar2=None,
                                        op0=alu.mult)
                nc.vector.scalar_tensor_tensor(out=h1, in0=w32, scalar=SW, in1=h0,
                                               op0=alu.mult, op1=alu.subtract)
                wsb[name] = (h0, h1)

        rows = {}
        for name, rd in (("bq", bq_d), ("bk", bk_d), ("gq", gq_d), ("gk", gk_d)):
            rt = const.tile([1, A], f32, tag=f"{name}row")
            nc.sync.dma_start(out=rt, in_=rd[:])
            rows[name] = rt

        # gg = gq*gk as a row; bounce rows through DRAM into [128, 2] columns
        gg_row = const.tile([1, A], f32, tag="ggrow")
        nc.vector.scalar_tensor_tensor(out=gg_row, in0=rows["gq"], scalar=0.0,
                                       in1=rows["gk"], op0=alu.bypass, op1=alu.mult)
        cols = {}
        for name, rt in (("gg", gg_row), ("bq", rows["bq"]), ("bk", rows["bk"])):
            cd = dram.tile([1, A], f32, tag=f"{name}dram")
            nc.gpsimd.dma_start(out=cd[:], in_=rt[:])
            ct = const.tile([P, 2], f32, tag=f"{name}col")
            nc.gpsimd.dma_start(out=ct,
                                in_=cd[:].rearrange("o (a p) -> p (o a)", p=P))
            cols[name] = ct

        bv_bc = const.tile([P, C], f32, tag="bvbc")
        nc.gpsimd.dma_start(out=bv_bc, in_=bv_d[:].to_broadcast([P, C]))
        ones_col = const.tile([P, 1], f32, tag="ones")
        nc.vector.memset(ones_col, 1.0)

        v_dram = dram.tile([NCODES, C], f32)

        for rep in range(reps):
            prep_ctx = ExitStack()
            ppool = lambda name, bufs: prep_ctx.enter_context(
                tc.tile_pool(name=f"{name}_{rep}", bufs=bufs, space="SBUF")
            )
            chp = ppool("chp", 4)       # fp32 input chunks streamed from DRAM
            inpair = ppool("inpair", 6)  # per-chunk fp16 pairs of the input
            krawp = ppool("kraw", 1)
            qrawp = ppool("qraw", 1)
            sqp = ppool("sqp", 3)       # per-chunk (raw+bias)^2 from PSUM
            prscr = ppool("prscr", 2)   # pair-construction fp32 scratch
            bcch = ppool("bcch", 2)     # per-chunk rsqrt broadcast
            rowp = ppool("rowp", 2)     # [1, 512] sumsq rows
            n8x = ppool("n8x", 2)
            n8y = ppool("n8y", 2)
            n8a = ppool("n8a", 1)
            vsb = ppool("vsb", 3)
            prodk_p = ppool("prodk", 2)  # whole-tensor k-tilde hi/lo
            prodq_p = ppool("prodq", 16)  # per-chunk q-tilde hi/lo (all live)

            def in_pairs(dram_ap, sl):
                ch32 = chp.tile([P, 2, NCH], f32, tag="ch")
                nc.sync.dma_start(out=ch32, in_=dram_ap[:, :, sl])
                h0 = inpair.tile([P, 2, NCH], f16, tag="iph")
                h1 = inpair.tile([P, 2, NCH], f16, tag="iph")
                nc.vector.tensor_scalar(out=h0, in0=ch32, scalar1=SIN,
                                        scalar2=None, op0=alu.mult)
                nc.vector.scalar_tensor_tensor(out=h1, in0=ch32, scalar=SIN,
                                               in1=h0, op0=alu.mult,
                                               op1=alu.subtract)
                return h0, h1

            def proj_mms(ps, wpair, xpair, dt_):
                wh0, wh1 = wpair
                xh0, xh1 = xpair
                dsl = slice(dt_ * P, (dt_ + 1) * P)
                mms = [(lw, ht, rx) for ht in range(2)
                       for lw, rx in ((wh0, xh0), (wh0, xh1), (wh1, xh0))]
                for i, (lw, ht, rx) in enumerate(mms):
                    nc.tensor.matmul(ps, lhsT=lw[:, ht, dsl], rhs=rx[:, ht, :],
                                     start=(i == 0), stop=(i == len(mms) - 1))

            def side_proj_mk(dram_ap, wname, bcol, XW, with_v, rawpool):
                raw_sb = rawpool.tile([P, 2, XW], f32, tag="raw", name="raw_sb")
                x_dram = dram.tile([1, XW], f32, tag=f"x_dram_{wname}_{rep}",
                                   name="x_dram")
                state = {"pending": None}

                def emit_sumsq(c, sq):
                    sl_ = slice(c * NCH, (c + 1) * NCH)
                    psr = ps_row.tile([1, NCH], f32, tag="ps_row")
                    nc.tensor.matmul(psr, lhsT=ones_col, rhs=sq[:, 0, :],
                                     start=True, stop=False)
                    nc.tensor.matmul(psr, lhsT=ones_col, rhs=sq[:, 1, :],
                                     start=False, stop=True)
                    xc = rowp.tile([1, NCH], f32, tag="xrow")
                    nc.scalar.activation(xc, psr, Copy)
                    nc.gpsimd.dma_start(out=x_dram[0:1, sl_], in_=xc)

                def emit_chunk(c):
                    sl = slice(c * NCH, (c + 1) * NCH)
                    pair = in_pairs(dram_ap, sl)
                    sq = sqp.tile([P, 2, NCH], f32, tag="sq")
                    for dt_ in range(2):
                        ps = ps_big.tile([P, NCH], f32, tag="ps_big")
                        proj_mms(ps, wsb[wname], pair, dt_)
                        nc.scalar.activation(raw_sb[:, dt_, sl], ps, Ident,
                                             bias=bcol[:, dt_:dt_ + 1],
                                             scale=DPROJ)
                        nc.scalar.activation(sq[:, dt_, :], ps, Square,
                                             bias=bcol[:, dt_:dt_ + 1],
                                             scale=DPROJ)
                    if state["pending"] is not None:
                        emit_sumsq(*state["pending"])
                    state["pending"] = (c, sq)
                    if with_v:
                        for jj in range(NCH // P):
                            j = c * (NCH // P) + jj
                            nsl = slice(jj * P, (jj + 1) * P)
                            psv = ps_v.tile([P, C], f32, tag="ps_v")
                            vm = [(lc, ht, rw) for ht in range(2)
                                  for lc, rw in ((pair[0], wsb["wv"][0]),
                                                 (pair[0], wsb["wv"][1]),
                                                 (pair[1], wsb["wv"][0]))]
                            for i, (lc, ht, rw) in enumerate(vm):
                                nc.tensor.matmul(psv, lhsT=lc[:, ht, nsl],
                                                 rhs=rw[:, ht, :], start=(i == 0),
                                                 stop=(i == len(vm) - 1))
                            v_sb = vsb.tile([P, C], f32, tag="v_sb")
                            nc.vector.scalar_tensor_tensor(
                                out=v_sb, in0=psv, scalar=DPROJ,
                                in1=bv_bc, op0=alu.mult, op1=alu.add)
                            nc.sync.dma_start(out=v_dram[j * P:(j + 1) * P, :],
                                              in_=v_sb)

                def flush():
                    if state["pending"] is not None:
                        emit_sumsq(*state["pending"])
                        state["pending"] = None

                return raw_sb, x_dram, emit_chunk, flush

            def rsqrt_rows(x_dram, XW, act_scale, act_bias, tag):
                nch8 = XW // NCH
                x8f = n8x.tile([8, NCH], f32, tag="x8", name="x8f")
                nc.gpsimd.dma_start(
                    out=x8f[:nch8, :],
                    in_=x_dram[:].rearrange("o (a n) -> (o a) n", a=nch8))
                xs = x8f[:nch8, :]
                xbf = n8x.tile([8, NCH], f32, tag="x8", name="xbf")
                xb = xbf[:nch8, :]
                nc.vector.tensor_scalar(out=xb, in0=xs, scalar1=act_scale,
                                        scalar2=act_bias, op0=alu.mult,
                                        op1=alu.add)
                y0 = n8y.tile([8, NCH], f32, tag="y", name="y0")
                y = y0[:nch8, :]
                nc.scalar.activation(y, xb, Sqrt)
                nc.vector.reciprocal(out=y, in_=y)
                for _ in range(2):  # y <- y * (1.5 - 0.5 * xb * y^2)
                    a8f = n8a.tile([8, NCH], f32, tag="a8", name="a8f")
                    a8 = a8f[:nch8, :]
                    nc.vector.scalar_tensor_tensor(
                        out=a8, in0=xb, scalar=0.0, in1=y,
                        op0=alu.bypass, op1=alu.mult)
                    nc.vector.scalar_tensor_tensor(
                        out=a8, in0=a8, scalar=-0.5, in1=y,
                        op0=alu.mult, op1=alu.mult)
                    nc.vector.tensor_scalar(out=a8, in0=a8, scalar1=1.5,
                                            scalar2=None, op0=alu.add)
                    ynf = n8y.tile([8, NCH], f32, tag="y", name="ynf")
                    yn = ynf[:nch8, :]
                    nc.vector.scalar_tensor_tensor(
                        out=yn, in0=y, scalar=0.0, in1=a8,
                        op0=alu.bypass, op1=alu.mult)
                    y = yn
                r_dram = dram.tile([1, XW], f32, tag=f"r_dram_{tag}_{rep}")
                nc.gpsimd.dma_start(
                    out=r_dram[:].rearrange("o (a n) -> (o a) n", a=nch8), in_=y)
                return r_dram

            def make_prod_pair(raw_sb, r_dram, XW, col_scalar, tag, per_chunk):
                out_pairs = []
                if not per_chunk:
                    h0t = prodk_p.tile([P, 2, XW], f16, tag=tag, name="h0t")
                    h1t = prodk_p.tile([P, 2, XW], f16, tag=tag, name="h1t")
                for c in range(XW // NCH):
                    sl = slice(c * NCH, (c + 1) * NCH)
                    if per_chunk:
                        h0t = prodq_p.tile([P, 2, NCH], f16, tag=tag, name="h0c")
                        h1t = prodq_p.tile([P, 2, NCH], f16, tag=tag, name="h1c")
                        osl = slice(0, NCH)
                        out_pairs.append((h0t, h1t))
                    else:
                        osl = sl
                    rc = bcch.tile([P, NCH], f32, tag="bcch")
                    nc.sync.dma_start(out=rc,
                                      in_=r_dram[0:1, sl].to_broadcast([P, NCH]))
                    for dt_ in range(2):
                        scr = prscr.tile([P, NCH], f32, tag="prscr")
                        if col_scalar is not None:
                            nc.vector.scalar_tensor_tensor(
                                out=scr, in0=raw_sb[:, dt_, sl],
                                scalar=col_scalar[:, dt_:dt_ + 1],
                                in1=rc, op0=alu.mult, op1=alu.mult)
                        else:
                            nc.vector.scalar_tensor_tensor(
                                out=scr, in0=raw_sb[:, dt_, sl], scalar=0.0,
                                in1=rc, op0=alu.bypass, op1=alu.mult)
                        nc.vector.tensor_copy(out=h0t[:, dt_, osl], in_=scr)
                        nc.vector.scalar_tensor_tensor(
                            out=h1t[:, dt_, osl], in0=scr, scalar=0.0,
                            in1=h0t[:, dt_, osl], op0=alu.bypass,
                            op1=alu.subtract)
                return out_pairs if per_chunk else (h0t, h1t)

            # ---- k side (with v) fully, then q side ----
            k_raw, xk_dram, k_emit, k_flush = side_proj_mk(
                cbT_r, "wk", cols["bk"], NCODES, True, krawp)
            q_raw, xq_dram, q_emit, q_flush = side_proj_mk(
                hsT_r, "wq", cols["bq"], T, False, qrawp)
            for c in range(NCHUNKS):
                k_emit(c)
            k_flush()
            for c in range(NCHUNKS):
                q_emit(c)
            q_flush()
            # rk' = SK*rsqrt(sumsq/A+eps) == rsqrt((sumsq/A+eps)/SK^2)
            rk_dram = rsqrt_rows(xk_dram, NCODES, 1.0 / (A * SK * SK),
                                 EPS / (SK * SK), "k")
            # fq' = SQ*(1/sqrt(A))*rsqrt(sumsq/A+eps) == rsqrt((sumsq+A*eps)/SQ^2)
            fq_dram = rsqrt_rows(xq_dram, T, 1.0 / (SQ * SQ),
                                 A * EPS / (SQ * SQ), "q")
            ktil = make_prod_pair(k_raw, rk_dram, NCODES, cols["gg"],
                                  "prodk", False)
            qtil_c = make_prod_pair(q_raw, fq_dram, T, None, "prodq", True)

            prep_ctx.close()
            mctx = ExitStack()
            mpool = lambda name, bufs: mctx.enter_context(
                tc.tile_pool(name=f"{name}_{rep}", bufs=bufs, space="SBUF")
            )
            lgp = mpool("lgp", 3)
            iota_bc = mpool("iotap", 1).tile([P, NCODES], f32, tag="iota")
            nc.gpsimd.dma_start(out=iota_bc,
                                in_=iota_d[:].to_broadcast([P, NCODES]))
            scrp = mpool("scrp", 2)
            smal = mpool("smal", 4)
            zqp = mpool("zqp", 3)
            idx_all = mpool("idxall", 1).tile([P, TT], i32, tag="idxall")

            for tt in range(TT):
                tsl = slice(tt * P, (tt + 1) * P)
                lg = lgp.tile([P, NCODES], f32, tag="lg")
                qtil = qtil_c[tt // 4]
                qsl = slice((tt % 4) * P, (tt % 4 + 1) * P)
                for c in range(NCHUNKS):
                    sl = slice(c * NCH, (c + 1) * NCH)
                    ps = ps_big.tile([P, NCH], f32, tag="ps_big")
                    bmms = [(lq, dt_, rk_) for dt_ in range(2)
                            for lq, rk_ in ((qtil[0], ktil[0]),
                                            (qtil[0], ktil[1]),
                                            (qtil[1], ktil[0]))]
                    for i, (lq, dt_, rk_) in enumerate(bmms):
                        nc.tensor.matmul(ps, lhsT=lq[:, dt_, qsl],
                                         rhs=rk_[:, dt_, sl],
                                         start=(i == 0),
                                         stop=(i == len(bmms) - 1))
                    nc.scalar.activation(lg[:, sl], ps, Copy, scale=DBIG)
                nc.sync.dma_start(out=logits_o[tsl, :], in_=lg)

                # argmax: pairwise max tree, then (lg == m) * iota with
                # free-dim sum accumulation (split DVE/GPSIMD by tile parity)
                mx4 = scrp.tile([P, 4 * NCH], f32, tag="mx4")
                for i in range(4):
                    nc.vector.scalar_tensor_tensor(
                        out=mx4[:, i * NCH:(i + 1) * NCH],
                        in0=lg[:, (2 * i) * NCH:(2 * i + 1) * NCH], scalar=0.0,
                        in1=lg[:, (2 * i + 1) * NCH:(2 * i + 2) * NCH],
                        op0=alu.bypass, op1=alu.max)
                mx2 = scrp.tile([P, 2 * NCH], f32, tag="mx2")
                for i in range(2):
                    nc.vector.scalar_tensor_tensor(
                        out=mx2[:, i * NCH:(i + 1) * NCH],
                        in0=mx4[:, (2 * i) * NCH:(2 * i + 1) * NCH], scalar=0.0,
                        in1=mx4[:, (2 * i + 1) * NCH:(2 * i + 2) * NCH],
                        op0=alu.bypass, op1=alu.max)
                mx1 = scrp.tile([P, NCH], f32, tag="mx1")
                nc.vector.scalar_tensor_tensor(
                    out=mx1, in0=mx2[:, :NCH], scalar=0.0, in1=mx2[:, NCH:],
                    op0=alu.bypass, op1=alu.max)
                m = smal.tile([P, 1], f32, tag="m")
                nc.vector.reduce_max(out=m, in_=mx1, axis=AX)

                eq_eng = nc.vector
                acc = smal.tile([P, NCHUNKS], f32, tag="acc")
                for c in range(NCHUNKS):
                    sl = slice(c * NCH, (c + 1) * NCH)
                    scr = scrp.tile([P, NCH], f32, tag="scr")
                    eq_eng.scalar_tensor_tensor(
                        out=scr, in0=lg[:, sl], scalar=m[:, 0:1],
                        in1=iota_bc[:, sl], op0=alu.is_equal, op1=alu.mult,
                        accum_out=acc[:, c:c + 1])
                idxf = smal.tile([P, 1], f32, tag="idxf")
                nc.vector.reduce_sum(out=idxf, in_=acc, axis=AX)
                nc.vector.tensor_copy(out=idx_all[:, tt:tt + 1], in_=idxf)

                zq = zqp.tile([P, C], f32, tag="zq")
                nc.gpsimd.indirect_dma_start(
                    out=zq[:, :], out_offset=None, in_=v_dram[:, :],
                    in_offset=bass.IndirectOffsetOnAxis(
                        ap=idx_all[:, tt:tt + 1], axis=0))
                nc.sync.dma_start(out=zq_o[tsl, :], in_=zq)
                nc.sync.dma_start(out=zq2_o[tsl, :], in_=zq)

            nc.gpsimd.dma_start(
                out=idx_o[:].rearrange("(a p) o -> p (a o)", p=P), in_=idx_all)
            mctx.close()

    if not nc.is_finalized():
        nc.finalize()
    return nc


_NC_CACHE = {}


def _get_nc(reps=1):
    if reps not in _NC_CACHE:
        _NC_CACHE[reps] = _build_module(reps)
    return _NC_CACHE[reps]


def kernel(hidden_states, codebook_hidden_states, wq, bq, wk, bk, wv, bv, gq, gk):
    from concourse.bass_utils import run_bass_kernel_spmd

    hs = np.ascontiguousarray(np.asarray(hidden_states, dtype=np.float32))
    cb = np.ascontiguousarray(np.asarray(codebook_hidden_states, dtype=np.float32))
    wq = np.ascontiguousarray(np.asarray(wq, dtype=np.float32))
    wk = np.ascontiguousarray(np.asarray(wk, dtype=np.float32))
    wv = np.ascontiguousarray(np.asarray(wv, dtype=np.float32))
    bq = np.asarray(bq, dtype=np.float32).reshape(1, A)
    bk = np.asarray(bk, dtype=np.float32).reshape(1, A)
    bv = np.asarray(bv, dtype=np.float32).reshape(1, C)
    gq = np.asarray(gq, dtype=np.float32).reshape(1, A)
    gk = np.asarray(gk, dtype=np.float32).reshape(1, A)

    hsT = np.ascontiguousarray(hs.T)          # [H, 32768]
    cbT = np.ascontiguousarray(cb.T)          # [H, 4096]
    iota = np.arange(NCODES, dtype=np.float32).reshape(1, NCODES)

    nc = _get_nc()
    in_maps = []
    for core in range(N_CORES):
        sl = slice(core * T, (core + 1) * T)
        in_maps.append({
            "hsT": np.ascontiguousarray(hsT[:, sl]),
            "cbT": cbT, "wq": wq, "wk": wk, "wv": wv,
            "bq": bq, "bk": bk, "bv": bv, "gq": gq, "gk": gk,
            "iota": iota,
        })

    res = run_bass_kernel_spmd(nc, in_maps, list(range(N_CORES)))
    kernel.last_results = res

    logits = np.concatenate([res.results[i]["logits"] for i in range(N_CORES)],
                            axis=0)
    idx = np.concatenate([res.results[i]["idx"] for i in range(N_CORES)], axis=0)
    zq = np.concatenate([res.results[i]["zq"] for i in range(N_CORES)], axis=0)
    zq2 = np.concatenate([res.results[i]["zq2"] for i in range(N_CORES)], axis=0)
    return (logits, idx.astype(np.int32), zq, zq2)


# revision 40
# speedup vs baseline: 1.0183x; 1.0183x over previous
"""Trainium2 Bass kernel for nn_AttentionForQuantizer.

Computes, for hidden_states [32768, 256] and a 4096-entry codebook:
    q = rmsnorm(hs @ wq + bq) * gq ;  k = rmsnorm(cb @ wk + bk) * gk
    logits = (q @ k.T) / sqrt(A)
    idx    = argmax(logits, axis=1)
    z_q = z_q_2 = v[idx]   where v = cb @ wv + bv
(the softmax in the reference cancels out of every returned tensor up to
~1 ulp, so it is never computed).

Sharding: data-parallel over the token axis across 8 NeuronCores; codebook
and weights replicated.  Math identity used on device:
    logits[t, n] = fq[t] * sum_d q_raw[t, d] * gg[d] * rk[n] * k_raw[n, d]
with fq[t] = scale * rsqrt(mean_d q_raw^2 + eps), rk[n] = rsqrt(mean + eps),
gg = gq * gk.  fq folds into the lhsT operand, gg/rk into the rhs operand.

All accuracy-critical matmuls run as fp16 hi/lo pairs with fp32 PSUM
accumulation (3 passes: h0k0 + h0k1 + h1k0; the dropped h1k1 term is
O(2^-22)).  Measured deviation vs float64 is ~3.5e-6 absolute on the logits
-- below plain fp32 matmul reordering noise and 10x below the dataset's
minimum top1-top2 logit gap, so argmax is stable.  Power-of-2 scales keep
the lo parts out of fp16-subnormal range; descales fold into the
(mandatory) PSUM->SBUF eviction ops for free.
"""

import os
import numpy as np

P = 128
T = 4096          # tokens per core
NCODES = 4096     # codebook entries
H = 256           # hidden dim (2 partition tiles)
A = 256           # attn dim   (2 partition tiles)
C = 256           # v output dim
NCH = 512         # n-chunk width (1 PSUM bank of fp32)
NCHUNKS = NCODES // NCH
TT = T // P       # token tiles per core
EPS = 1e-5
N_CORES = 8

SIN = 64.0        # input (hs/cb) scale for fp16 pairs
SW = 128.0        # weight scale
DPROJ = 1.0 / (SIN * SW)      # projection descale 2^-13
SQ = 128.0        # extra scale folded into fq
SK = 32.0         # extra scale folded into rk
DBIG = 1.0 / (SQ * SK)        # big-matmul descale 2^-12


def _build_module(reps=1):
    import concourse.bass as bass
    from concourse import bacc, mybir
    from concourse.tile import TileContext
    from contextlib import ExitStack

    f32 = mybir.dt.float32
    f16 = mybir.dt.float16
    i32 = mybir.dt.int32
    Copy = mybir.ActivationFunctionType.Copy
    Ident = mybir.ActivationFunctionType.Identity
    Sqrt = mybir.ActivationFunctionType.Sqrt
    Square = mybir.ActivationFunctionType.Square
    alu = mybir.AluOpType
    AX = mybir.AxisListType.X

    nc = bacc.Bacc()

    # ---- I/O ----
    hsT_d = nc.declare_dram_parameter("hsT", [H, T], f32, isOutput=False)
    cbT_d = nc.declare_dram_parameter("cbT", [H, NCODES], f32, isOutput=False)
    wq_d = nc.declare_dram_parameter("wq", [H, A], f32, isOutput=False)
    wk_d = nc.declare_dram_parameter("wk", [H, A], f32, isOutput=False)
    wv_d = nc.declare_dram_parameter("wv", [H, C], f32, isOutput=False)
    bq_d = nc.declare_dram_parameter("bq", [1, A], f32, isOutput=False)
    bk_d = nc.declare_dram_parameter("bk", [1, A], f32, isOutput=False)
    bv_d = nc.declare_dram_parameter("bv", [1, C], f32, isOutput=False)
    gq_d = nc.declare_dram_parameter("gq", [1, A], f32, isOutput=False)
    gk_d = nc.declare_dram_parameter("gk", [1, A], f32, isOutput=False)
    iota_d = nc.declare_dram_parameter("iota", [1, NCODES], f32, isOutput=False)

    logits_o = nc.declare_dram_parameter("logits", [T, NCODES], f32, isOutput=True)
    idx_o = nc.declare_dram_parameter("idx", [T, 1], i32, isOutput=True)
    zq_o = nc.declare_dram_parameter("zq", [T, C], f32, isOutput=True)
    zq2_o = nc.declare_dram_parameter("zq2", [T, C], f32, isOutput=True)

    hsT_r = hsT_d[:].rearrange("(a p) t -> p a t", p=P)
    cbT_r = cbT_d[:].rearrange("(a p) n -> p a n", p=P)

    with TileContext(nc) as tc, ExitStack() as ctx:
        pool = lambda name, bufs, space="SBUF": ctx.enter_context(
            tc.tile_pool(name=name, bufs=bufs, space=space)
        )

        const = pool("const", 1)
        dram = pool("dram", 1, "DRAM")
        ps_big = pool("ps_big", 4, "PSUM")

        # ---- weights: scaled fp16 hi/lo pairs (scratch released after) ----
        wsb = {}
        with tc.tile_pool(name="wtmp", bufs=2) as wtmp:
            for name, wd in (("wq", wq_d), ("wk", wk_d), ("wv", wv_d)):
                w32 = wtmp.tile([P, 2, A], f32, tag="w32")
                nc.sync.dma_start(out=w32,
                                  in_=wd[:].rearrange("(a p) d -> p a d", p=P))
                h0 = const.tile([P, 2, A], f16, tag=f"{name}h0")
                h1 = const.tile([P, 2, A], f16, tag=f"{name}h1")
                nc.vector.tensor_scalar(out=h0, in0=w32, scalar1=SW, scalar2=None,
                                        op0=alu.mult)
                nc.vector.scalar_tensor_tensor(out=h1, in0=w32, scalar=SW, in1=h0,
                                               op0=alu.mult, op1=alu.subtract)
                wsb[name] = (h0, h1)

        rows = {}
        for name, rd in (("bq", bq_d), ("bk", bk_d), ("gq", gq_d), ("gk", gk_d)):
            rt = const.tile([1, A], f32, tag=f"{name}row")
            nc.sync.dma_start(out=rt, in_=rd[:])
            rows[name] = rt

        # gg = gq*gk as a row; bounce rows through DRAM into [128, 2] columns
        gg_row = const.tile([1, A], f32, tag="ggrow")
        nc.vector.scalar_tensor_tensor(out=gg_row, in0=rows["gq"], scalar=0.0,
                                       in1=rows["gk"], op0=alu.bypass, op1=alu.mult)
        cols = {}
        for name, rt in (("gg", gg_row), ("bq", rows["bq"]), ("bk", rows["bk"])):
            cd = dram.tile([1, A], f32, tag=f"{name}dram")
            nc.gpsimd.dma_start(out=cd[:], in_=rt[:])
            ct = const.tile([P, 2], f32, tag=f"{name}col")
            nc.gpsimd.dma_start(out=ct,
                                in_=cd[:].rearrange("o (a p) -> p (o a)", p=P))
            cols[name] = ct

        bv_bc = const.tile([P, C], f32, tag="bvbc")
        nc.gpsimd.dma_start(out=bv_bc, in_=bv_d[:].to_broadcast([P, C]))
        ones_col = const.tile([P, 1], f32, tag="ones")
        nc.vector.memset(ones_col, 1.0)

        v_dram = dram.tile([NCODES, C], f32)

        for rep in range(reps):
            prep_ctx = ExitStack()
            ppool = lambda name, bufs: prep_ctx.enter_context(
                tc.tile_pool(name=f"{name}_{rep}", bufs=bufs, space="SBUF")
            )
            chp = ppool("chp", 4)       # fp32 input chunks streamed from DRAM
            inpair = ppool("inpair", 6)  # per-chunk fp16 pairs of the input
            krawp = ppool("kraw", 1)
            qrawp = ppool("qraw", 1)
            sqp = ppool("sqp", 3)       # per-chunk (raw+bias)^2 from PSUM
            prscr = ppool("prscr", 2)   # pair-construction fp32 scratch
            bcch = ppool("bcch", 2)     # per-chunk rsqrt broadcast
            rowp = ppool("rowp", 2)     # [1, 512] sumsq rows
            n8x = ppool("n8x", 2)
            n8y = ppool("n8y", 2)
            n8a = ppool("n8a", 1)
            vsb = ppool("vsb", 3)
            prodk_p = ppool("prodk", 2)  # whole-tensor k-tilde hi/lo
            prodq_p = ppool("prodq", 16)  # per-chunk q-tilde hi/lo (all live)
            ps_v = prep_ctx.enter_context(
                tc.tile_pool(name=f"ps_v_{rep}", bufs=2, space="PSUM"))
            ps_row = prep_ctx.enter_context(
                tc.tile_pool(name=f"ps_row_{rep}", bufs=2, space="PSUM"))

            def in_pairs(dram_ap, sl):
                ch32 = chp.tile([P, 2, NCH], f32, tag="ch")
                nc.sync.dma_start(out=ch32, in_=dram_ap[:, :, sl])
                h0 = inpair.tile([P, 2, NCH], f16, tag="iph")
                h1 = inpair.tile([P, 2, NCH], f16, tag="iph")
                nc.vector.tensor_scalar(out=h0, in0=ch32, scalar1=SIN,
                                        scalar2=None, op0=alu.mult)
                nc.vector.scalar_tensor_tensor(out=h1, in0=ch32, scalar=SIN,
                                               in1=h0, op0=alu.mult,
                                               op1=alu.subtract)
                return h0, h1

            def proj_mms(ps, wpair, xpair, dt_):
                wh0, wh1 = wpair
                xh0, xh1 = xpair
                dsl = slice(dt_ * P, (dt_ + 1) * P)
                mms = [(lw, ht, rx) for ht in range(2)
                       for lw, rx in ((wh0, xh0), (wh0, xh1), (wh1, xh0))]
                for i, (lw, ht, rx) in enumerate(mms):
                    nc.tensor.matmul(ps, lhsT=lw[:, ht, dsl], rhs=rx[:, ht, :],
                                     start=(i == 0), stop=(i == len(mms) - 1))

            def side_proj_mk(dram_ap, wname, bcol, XW, with_v, rawpool):
                raw_sb = rawpool.tile([P, 2, XW], f32, tag="raw", name="raw_sb")
                x_dram = dram.tile([1, XW], f32, tag=f"x_dram_{wname}_{rep}",
                                   name="x_dram")
                state = {"pending": None}

                def emit_sumsq(c, sq):
                    sl_ = slice(c * NCH, (c + 1) * NCH)
                    psr = ps_row.tile([1, NCH], f32, tag="ps_row")
                    nc.tensor.matmul(psr, lhsT=ones_col, rhs=sq[:, 0, :],
                                     start=True, stop=False)
                    nc.tensor.matmul(psr, lhsT=ones_col, rhs=sq[:, 1, :],
                                     start=False, stop=True)
                    xc = rowp.tile([1, NCH], f32, tag="xrow")
                    nc.scalar.activation(xc, psr, Copy)
                    nc.gpsimd.dma_start(out=x_dram[0:1, sl_], in_=xc)

                def emit_chunk(c):
                    sl = slice(c * NCH, (c + 1) * NCH)
                    pair = in_pairs(dram_ap, sl)
                    sq = sqp.tile([P, 2, NCH], f32, tag="sq")
                    for dt_ in range(2):
                        ps = ps_big.tile([P, NCH], f32, tag="ps_big")
                        proj_mms(ps, wsb[wname], pair, dt_)
                        nc.scalar.activation(raw_sb[:, dt_, sl], ps, Ident,
                                             bias=bcol[:, dt_:dt_ + 1],
                                             scale=DPROJ)
                        nc.scalar.activation(sq[:, dt_, :], ps, Square,
                                             bias=bcol[:, dt_:dt_ + 1],
                                             scale=DPROJ)
                    if state["pending"] is not None:
                        emit_sumsq(*state["pending"])
                    state["pending"] = (c, sq)
                    if with_v:
                        for jj in range(NCH // P):
                            j = c * (NCH // P) + jj
                            nsl = slice(jj * P, (jj + 1) * P)
                            psv = ps_v.tile([P, C], f32, tag="ps_v")
                            vm = [(lc, ht, rw) for ht in range(2)
                                  for lc, rw in ((pair[0], wsb["wv"][0]),
                                                 (pair[0], wsb["wv"][1]),
                                                 (pair[1], wsb["wv"][0]))]
                            for i, (lc, ht, rw) in enumerate(vm):
                                nc.tensor.matmul(psv, lhsT=lc[:, ht, nsl],
                                                 rhs=rw[:, ht, :], start=(i == 0),
                                                 stop=(i == len(vm) - 1))
                            v_sb = vsb.tile([P, C], f32, tag="v_sb")
                            nc.vector.scalar_tensor_tensor(
                                out=v_sb, in0=psv, scalar=DPROJ,
                                in1=bv_bc, op0=alu.mult, op1=alu.add)
                            nc.sync.dma_start(out=v_dram[j * P:(j + 1) * P, :],
                                              in_=v_sb)

                def flush():
                    if state["pending"] is not None:
                        emit_sumsq(*state["pending"])
                        state["pending"] = None

                return raw_sb, x_dram, emit_chunk, flush

            def rsqrt_rows(x_dram, XW, act_scale, act_bias, tag):
                nch8 = XW // NCH
                x8f = n8x.tile([8, NCH], f32, tag="x8", name="x8f")
                nc.gpsimd.dma_start(
                    out=x8f[:nch8, :],
                    in_=x_dram[:].rearrange("o (a n) -> (o a) n", a=nch8))
                xs = x8f[:nch8, :]
                xbf = n8x.tile([8, NCH], f32, tag="x8", name="xbf")
                xb = xbf[:nch8, :]
                nc.vector.tensor_scalar(out=xb, in0=xs, scalar1=act_scale,
                                        scalar2=act_bias, op0=alu.mult,
                                        op1=alu.add)
                y0 = n8y.tile([8, NCH], f32, tag="y", name="y0")
                y = y0[:nch8, :]
                nc.scalar.activation(y, xb, Sqrt)
                nc.vector.reciprocal(out=y, in_=y)
                for _ in range(2):  # y <- y * (1.5 - 0.5 * xb * y^2)
                    a8f = n8a.tile([8, NCH], f32, tag="a8", name="a8f")
                    a8 = a8f[:nch8, :]
                    nc.vector.scalar_tensor_tensor(
                        out=a8, in0=xb, scalar=0.0, in1=y,
                        op0=alu.bypass, op1=alu.mult)
                    nc.vector.scalar_tensor_tensor(
                        out=a8, in0=a8, scalar=-0.5, in1=y,
                        op0=alu.mult, op1=alu.mult)
                    nc.vector.tensor_scalar(out=a8, in0=a8, scalar1=1.5,
                                            scalar2=None, op0=alu.add)
                    ynf = n8y.tile([8, NCH], f32, tag="y", name="ynf")
                    yn = ynf[:nch8, :]
                    nc.vector.scalar_tensor_tensor(
                        out=yn, in0=y, scalar=0.0, in1=a8,
                        op0=alu.bypass, op1=alu.mult)
                    y = yn
                r_dram = dram.tile([1, XW], f32, tag=f"r_dram_{tag}_{rep}")
                nc.gpsimd.dma_start(
                    out=r_dram[:].rearrange("o (a n) -> (o a) n", a=nch8), in_=y)
                return r_dram

            def make_prod_pair(raw_sb, r_dram, XW, col_scalar, tag, per_chunk):
                out_pairs = []
                if not per_chunk:
                    h0t = prodk_p.tile([P, 2, XW], f16, tag=tag, name="h0t")
                    h1t = prodk_p.tile([P, 2, XW], f16, tag=tag, name="h1t")
                for c in range(XW // NCH):
                    sl = slice(c * NCH, (c + 1) * NCH)
                    if per_chunk:
                        h0t = prodq_p.tile([P, 2, NCH], f16, tag=tag, name="h0c")
                        h1t = prodq_p.tile([P, 2, NCH], f16, tag=tag, name="h1c")
                        osl = slice(0, NCH)
                        out_pairs.append((h0t, h1t))
                    else:
                        osl = sl
                    rc = bcch.tile([P, NCH], f32, tag="bcch")
                    nc.sync.dma_start(out=rc,
                                      in_=r_dram[0:1, sl].to_broadcast([P, NCH]))
                    for dt_ in range(2):
                        scr = prscr.tile([P, NCH], f32, tag="prscr")
                        if col_scalar is not None:
                            nc.vector.scalar_tensor_tensor(
                                out=scr, in0=raw_sb[:, dt_, sl],
                                scalar=col_scalar[:, dt_:dt_ + 1],
                                in1=rc, op0=alu.mult, op1=alu.mult)
                        else:
                            nc.vector.scalar_tensor_tensor(
                                out=scr, in0=raw_sb[:, dt_, sl], scalar=0.0,
                                in1=rc, op0=alu.bypass, op1=alu.mult)
                        nc.vector.tensor_copy(out=h0t[:, dt_, osl], in_=scr)
                        nc.vector.scalar_tensor_tensor(
                            out=h1t[:, dt_, osl], in0=scr, scalar=0.0,
                            in1=h0t[:, dt_, osl], op0=alu.bypass,
                            op1=alu.subtract)
                return out_pairs if per_chunk else (h0t, h1t)

            # ---- k side (with v) fully, then q side ----
            k_raw, xk_dram, k_emit, k_flush = side_proj_mk(
                cbT_r, "wk", cols["bk"], NCODES, True, krawp)
            q_raw, xq_dram, q_emit, q_flush = side_proj_mk(
                hsT_r, "wq", cols["bq"], T, False, qrawp)
            for c in range(NCHUNKS):
                k_emit(c)
            k_flush()
            for c in range(NCHUNKS):
                q_emit(c)
            q_flush()
            # rk' = SK*rsqrt(sumsq/A+eps) == rsqrt((sumsq/A+eps)/SK^2)
            rk_dram = rsqrt_rows(xk_dram, NCODES, 1.0 / (A * SK * SK),
                                 EPS / (SK * SK), "k")
            # fq' = SQ*(1/sqrt(A))*rsqrt(sumsq/A+eps) == rsqrt((sumsq+A*eps)/SQ^2)
            fq_dram = rsqrt_rows(xq_dram, T, 1.0 / (SQ * SQ),
                                 A * EPS / (SQ * SQ), "q")
            ktil = make_prod_pair(k_raw, rk_dram, NCODES, cols["gg"],
                                  "prodk", False)
            qtil_c = make_prod_pair(q_raw, fq_dram, T, None, "prodq", True)

            prep_ctx.close()
            mctx = ExitStack()
            mpool = lambda name, bufs: mctx.enter_context(
                tc.tile_pool(name=f"{name}_{rep}", bufs=bufs, space="SBUF")
            )
            lgp = mpool("lgp", 3)
            ps_big2 = mctx.enter_context(
                tc.tile_pool(name=f"ps_big2_{rep}", bufs=4, space="PSUM"))
            iota_bc = mpool("iotap", 1).tile([P, NCODES], f32, tag="iota")
            nc.gpsimd.dma_start(out=iota_bc,
                                in_=iota_d[:].to_broadcast([P, NCODES]))
            scrp = mpool("scrp", 2)
            smal = mpool("smal", 4)
            zqp = mpool("zqp", 3)
            idx_all = mpool("idxall", 1).tile([P, TT], i32, tag="idxall")

            for tt in range(TT):
                tsl = slice(tt * P, (tt + 1) * P)
                lg = lgp.tile([P, NCODES], f32, tag="lg")
                qtil = qtil_c[tt // 4]
                qsl = slice((tt % 4) * P, (tt % 4 + 1) * P)
                for c in range(NCHUNKS):
                    sl = slice(c * NCH, (c + 1) * NCH)
                    ps = (ps_big if c % 2 == 0 else ps_big2).tile(
                        [P, NCH], f32, tag="ps_big")
                    bmms = [(lq, dt_, rk_) for dt_ in range(2)
                            for lq, rk_ in ((qtil[0], ktil[0]),
                                            (qtil[0], ktil[1]),
                                            (qtil[1], ktil[0]))]
                    for i, (lq, dt_, rk_) in enumerate(bmms):
                        nc.tensor.matmul(ps, lhsT=lq[:, dt_, qsl],
                                         rhs=rk_[:, dt_, sl],
                                         start=(i == 0),
                                         stop=(i == len(bmms) - 1))
                    nc.scalar.activation(lg[:, sl], ps, Copy, scale=DBIG)
                nc.sync.dma_start(out=logits_o[tsl, :], in_=lg)

                # argmax: pairwise max tree, then (lg == m) * iota with
                # free-dim sum accumulation (split DVE/GPSIMD by tile parity)
                mx4 = scrp.tile([P, 4 * NCH], f32, tag="mx4")
                for i in range(4):
                    nc.vector.scalar_tensor_tensor(
                        out=mx4[:, i * NCH:(i + 1) * NCH],
                        in0=lg[:, (2 * i) * NCH:(2 * i + 1) * NCH], scalar=0.0,
                        in1=lg[:, (2 * i + 1) * NCH:(2 * i + 2) * NCH],
                        op0=alu.bypass, op1=alu.max)
                mx2 = scrp.tile([P, 2 * NCH], f32, tag="mx2")
                for i in range(2):
                    nc.vector.scalar_tensor_tensor(
                        out=mx2[:, i * NCH:(i + 1) * NCH],
                        in0=mx4[:, (2 * i) * NCH:(2 * i + 1) * NCH], scalar=0.0,
                        in1=mx4[:, (2 * i + 1) * NCH:(2 * i + 2) * NCH],
                        op0=alu.bypass, op1=alu.max)
                mx1 = scrp.tile([P, NCH], f32, tag="mx1")
                nc.vector.scalar_tensor_tensor(
                    out=mx1, in0=mx2[:, :NCH], scalar=0.0, in1=mx2[:, NCH:],
                    op0=alu.bypass, op1=alu.max)
                m = smal.tile([P, 1], f32, tag="m")
                nc.vector.reduce_max(out=m, in_=mx1, axis=AX)

                eq_eng = nc.vector
                acc = smal.tile([P, NCHUNKS], f32, tag="acc")
                for c in range(NCHUNKS):
                    sl = slice(c * NCH, (c + 1) * NCH)
                    scr = scrp.tile([P, NCH], f32, tag="scr")
                    eq_eng.scalar_tensor_tensor(
                        out=scr, in0=lg[:, sl], scalar=m[:, 0:1],
                        in1=iota_bc[:, sl], op0=alu.is_equal, op1=alu.mult,
                        accum_out=acc[:, c:c + 1])
                idxf = smal.tile([P, 1], f32, tag="idxf")
                nc.vector.reduce_sum(out=idxf, in_=acc, axis=AX)
                nc.vector.tensor_copy(out=idx_all[:, tt:tt + 1], in_=idxf)

                zq = zqp.tile([P, C], f32, tag="zq")
                nc.gpsimd.indirect_dma_start(
                    out=zq[:, :], out_offset=None, in_=v_dram[:, :],
                    in_offset=bass.IndirectOffsetOnAxis(
                        ap=idx_all[:, tt:tt + 1], axis=0))
                nc.sync.dma_start(out=zq_o[tsl, :], in_=zq)
                nc.sync.dma_start(out=zq2_o[tsl, :], in_=zq)

            nc.gpsimd.dma_start(
                out=idx_o[:].rearrange("(a p) o -> p (a o)", p=P), in_=idx_all)
            mctx.close()

    if not nc.is_finalized():
        nc.finalize()
    return nc


_NC_CACHE = {}


def _get_nc(reps=1):
    if reps not in _NC_CACHE:
        _NC_CACHE[reps] = _build_module(reps)
    return _NC_CACHE[reps]


def kernel(hidden_states, codebook_hidden_states, wq, bq, wk, bk, wv, bv, gq, gk):
    from concourse.bass_utils import run_bass_kernel_spmd

    hs = np.ascontiguousarray(np.asarray(hidden_states, dtype=np.float32))
    cb = np.ascontiguousarray(np.asarray(codebook_hidden_states, dtype=np.float32))
    wq = np.ascontiguousarray(np.asarray(wq, dtype=np.float32))
    wk = np.ascontiguousarray(np.asarray(wk, dtype=np.float32))
    wv = np.ascontiguousarray(np.asarray(wv, dtype=np.float32))
    bq = np.asarray(bq, dtype=np.float32).reshape(1, A)
    bk = np.asarray(bk, dtype=np.float32).reshape(1, A)
    bv = np.asarray(bv, dtype=np.float32).reshape(1, C)
    gq = np.asarray(gq, dtype=np.float32).reshape(1, A)
    gk = np.asarray(gk, dtype=np.float32).reshape(1, A)

    hsT = np.ascontiguousarray(hs.T)          # [H, 32768]
    cbT = np.ascontiguousarray(cb.T)          # [H, 4096]
    iota = np.arange(NCODES, dtype=np.float32).reshape(1, NCODES)

    nc = _get_nc()
    in_maps = []
    for core in range(N_CORES):
        sl = slice(core * T, (core + 1) * T)
        in_maps.append({
            "hsT": np.ascontiguousarray(hsT[:, sl]),
            "cbT": cbT, "wq": wq, "wk": wk, "wv": wv,
            "bq": bq, "bk": bk, "bv": bv, "gq": gq, "gk": gk,
            "iota": iota,
        })

    res = run_bass_kernel_spmd(nc, in_maps, list(range(N_CORES)))
    kernel.last_results = res

    logits = np.concatenate([res.results[i]["logits"] for i in range(N_CORES)],
                            axis=0)
    idx = np.concatenate([res.results[i]["idx"] for i in range(N_CORES)], axis=0)
    zq = np.concatenate([res.results[i]["zq"] for i in range(N_CORES)], axis=0)
    zq2 = np.concatenate([res.results[i]["zq2"] for i in range(N_CORES)], axis=0)
    return (logits, idx.astype(np.int32), zq, zq2)


# revision 46
# speedup vs baseline: 1.0247x; 1.0063x over previous
"""Trainium2 Bass kernel for nn_AttentionForQuantizer.

Computes, for hidden_states [32768, 256] and a 4096-entry codebook:
    q = rmsnorm(hs @ wq + bq) * gq ;  k = rmsnorm(cb @ wk + bk) * gk
    logits = (q @ k.T) / sqrt(A)
    idx    = argmax(logits, axis=1)
    z_q = z_q_2 = v[idx]   where v = cb @ wv + bv
(the softmax in the reference cancels out of every returned tensor up to
~1 ulp, so it is never computed).

Sharding: data-parallel over the token axis across 8 NeuronCores; codebook
and weights replicated.  Math identity used on device:
    logits[t, n] = fq[t] * sum_d q_raw[t, d] * gg[d] * rk[n] * k_raw[n, d]
with fq[t] = scale * rsqrt(mean_d q_raw^2 + eps), rk[n] = rsqrt(mean + eps),
gg = gq * gk.  fq folds into the lhsT operand, gg/rk into the rhs operand.

All accuracy-critical matmuls run as fp16 hi/lo pairs with fp32 PSUM
accumulation (3 passes: h0k0 + h0k1 + h1k0; the dropped h1k1 term is
O(2^-22)).  Measured deviation vs float64 is ~3.5e-6 absolute on the logits
-- below plain fp32 matmul reordering noise and 10x below the dataset's
minimum top1-top2 logit gap, so argmax is stable.  Power-of-2 scales keep
the lo parts out of fp16-subnormal range; descales fold into the
(mandatory) PSUM->SBUF eviction ops for free.
"""

import os
import numpy as np

P = 128
T = 4096          # tokens per core
NCODES = 4096     # codebook entries
H = 256           # hidden dim (2 partition tiles)
A = 256           # attn dim   (2 partition tiles)
C = 256           # v output dim
NCH = 512         # n-chunk width (1 PSUM bank of fp32)
NCHUNKS = NCODES // NCH
TT = T // P       # token tiles per core
EPS = 1e-5
N_CORES = 8

SIN = 64.0        # input (hs/cb) scale for fp16 pairs
SW = 128.0        # weight scale
DPROJ = 1.0 / (SIN * SW)      # projection descale 2^-13
SQ = 128.0        # extra scale folded into fq
SK = 32.0         # extra scale folded into rk
DBIG = 1.0 / (SQ * SK)        # big-matmul descale 2^-12


def _build_module(reps=1):
    import concourse.bass as bass
    from concourse import bacc, mybir
    from concourse.tile import TileContext
    from contextlib import ExitStack

    f32 = mybir.dt.float32
    f16 = mybir.dt.float16
    i32 = mybir.dt.int32
    Copy = mybir.ActivationFunctionType.Copy
    Ident = mybir.ActivationFunctionType.Identity
    Sqrt = mybir.ActivationFunctionType.Sqrt
    Square = mybir.ActivationFunctionType.Square
    alu = mybir.AluOpType
    AX = mybir.AxisListType.X

    nc = bacc.Bacc()

    # ---- I/O ----
    hsT_d = nc.declare_dram_parameter("hsT", [H, T], f32, isOutput=False)
    cbT_d = nc.declare_dram_parameter("cbT", [H, NCODES], f32, isOutput=False)
    wq_d = nc.declare_dram_parameter("wq", [H, A], f32, isOutput=False)
    wk_d = nc.declare_dram_parameter("wk", [H, A], f32, isOutput=False)
    wv_d = nc.declare_dram_parameter("wv", [H, C], f32, isOutput=False)
    bq_d = nc.declare_dram_parameter("bq", [1, A], f32, isOutput=False)
    bk_d = nc.declare_dram_parameter("bk", [1, A], f32, isOutput=False)
    bv_d = nc.declare_dram_parameter("bv", [1, C], f32, isOutput=False)
    gq_d = nc.declare_dram_parameter("gq", [1, A], f32, isOutput=False)
    gk_d = nc.declare_dram_parameter("gk", [1, A], f32, isOutput=False)
    iota_d = nc.declare_dram_parameter("iota", [1, NCODES], f32, isOutput=False)

    logits_o = nc.declare_dram_parameter("logits", [T, NCODES], f32, isOutput=True)
    idx_o = nc.declare_dram_parameter("idx", [T, 1], i32, isOutput=True)
    zq_o = nc.declare_dram_parameter("zq", [T, C], f32, isOutput=True)
    zq2_o = nc.declare_dram_parameter("zq2", [T, C], f32, isOutput=True)

    hsT_r = hsT_d[:].rearrange("(a p) t -> p a t", p=P)
    cbT_r = cbT_d[:].rearrange("(a p) n -> p a n", p=P)

    with TileContext(nc) as tc, ExitStack() as ctx:
        pool = lambda name, bufs, space="SBUF": ctx.enter_context(
            tc.tile_pool(name=name, bufs=bufs, space=space)
        )

        const = pool("const", 1)
        dram = pool("dram", 1, "DRAM")
        ps_big = pool("ps_big", 4, "PSUM")

        # ---- weights: scaled fp16 hi/lo pairs (scratch released after) ----
        wsb = {}
        with tc.tile_pool(name="wtmp", bufs=2) as wtmp:
            for name, wd in (("wq", wq_d), ("wk", wk_d), ("wv", wv_d)):
                w32 = wtmp.tile([P, 2, A], f32, tag="w32")
                nc.sync.dma_start(out=w32,
                                  in_=wd[:].rearrange("(a p) d -> p a d", p=P))
                h0 = const.tile([P, 2, A], f16, tag=f"{name}h0")
                h1 = const.tile([P, 2, A], f16, tag=f"{name}h1")
                nc.vector.tensor_scalar(out=h0, in0=w32, scalar1=SW, scalar2=None,
                                        op0=alu.mult)
                nc.vector.scalar_tensor_tensor(out=h1, in0=w32, scalar=SW, in1=h0,
                                               op0=alu.mult, op1=alu.subtract)
                wsb[name] = (h0, h1)

        rows = {}
        for name, rd in (("bq", bq_d), ("bk", bk_d), ("gq", gq_d), ("gk", gk_d)):
            rt = const.tile([1, A], f32, tag=f"{name}row")
            nc.sync.dma_start(out=rt, in_=rd[:])
            rows[name] = rt

        # gg = gq*gk as a row; bounce rows through DRAM into [128, 2] columns
        gg_row = const.tile([1, A], f32, tag="ggrow")
        nc.vector.scalar_tensor_tensor(out=gg_row, in0=rows["gq"], scalar=0.0,
                                       in1=rows["gk"], op0=alu.bypass, op1=alu.mult)
        cols = {}
        for name, rt in (("gg", gg_row), ("bq", rows["bq"]), ("bk", rows["bk"])):
            cd = dram.tile([1, A], f32, tag=f"{name}dram")
            nc.gpsimd.dma_start(out=cd[:], in_=rt[:])
            ct = const.tile([P, 2], f32, tag=f"{name}col")
            nc.gpsimd.dma_start(out=ct,
                                in_=cd[:].rearrange("o (a p) -> p (o a)", p=P))
            cols[name] = ct

        bv_bc = const.tile([P, C], f32, tag="bvbc")
        nc.gpsimd.dma_start(out=bv_bc, in_=bv_d[:].to_broadcast([P, C]))
        ones_col = const.tile([P, 1], f32, tag="ones")
        nc.vector.memset(ones_col, 1.0)

        v_dram = dram.tile([NCODES, C], f32)

        for rep in range(reps):
            prep_ctx = ExitStack()
            ppool = lambda name, bufs: prep_ctx.enter_context(
                tc.tile_pool(name=f"{name}_{rep}", bufs=bufs, space="SBUF")
            )
            chp = ppool("chp", 4)       # fp32 input chunks streamed from DRAM
            inpair = ppool("inpair", 6)  # per-chunk fp16 pairs of the input
            krawp = ppool("kraw", 1)
            qrawp = ppool("qraw", 1)
            sqp = ppool("sqp", 3)       # per-chunk (raw+bias)^2 from PSUM
            prscr = ppool("prscr", 2)   # pair-construction fp32 scratch
            bcch = ppool("bcch", 2)     # per-chunk rsqrt broadcast
            rowp = ppool("rowp", 2)     # [1, 512] sumsq rows
            n8x = ppool("n8x", 2)
            n8y = ppool("n8y", 2)
            n8a = ppool("n8a", 1)
            vsb = ppool("vsb", 3)
            prodk_p = ppool("prodk", 2)  # whole-tensor k-tilde hi/lo
            prodq_p = ppool("prodq", 16)  # per-chunk q-tilde hi/lo (all live)
            ps_v = prep_ctx.enter_context(
                tc.tile_pool(name=f"ps_v_{rep}", bufs=2, space="PSUM"))
            ps_row = prep_ctx.enter_context(
                tc.tile_pool(name=f"ps_row_{rep}", bufs=2, space="PSUM"))

            def in_pairs(dram_ap, sl):
                ch32 = chp.tile([P, 2, NCH], f32, tag="ch")
                nc.sync.dma_start(out=ch32, in_=dram_ap[:, :, sl])
                h0 = inpair.tile([P, 2, NCH], f16, tag="iph")
                h1 = inpair.tile([P, 2, NCH], f16, tag="iph")
                nc.vector.tensor_scalar(out=h0, in0=ch32, scalar1=SIN,
                                        scalar2=None, op0=alu.mult)
                nc.vector.scalar_tensor_tensor(out=h1, in0=ch32, scalar=SIN,
                                               in1=h0, op0=alu.mult,
                                               op1=alu.subtract)
                return h0, h1

            def proj_mms(ps, wpair, xpair, dt_):
                wh0, wh1 = wpair
                xh0, xh1 = xpair
                dsl = slice(dt_ * P, (dt_ + 1) * P)
                mms = [(lw, ht, rx) for ht in range(2)
                       for lw, rx in ((wh0, xh0), (wh0, xh1), (wh1, xh0))]
                for i, (lw, ht, rx) in enumerate(mms):
                    nc.tensor.matmul(ps, lhsT=lw[:, ht, dsl], rhs=rx[:, ht, :],
                                     start=(i == 0), stop=(i == len(mms) - 1))

            def side_proj_mk(dram_ap, wname, bcol, XW, with_v, rawpool):
                raw_sb = rawpool.tile([P, 2, XW], f32, tag="raw", name="raw_sb")
                x_dram = dram.tile([1, XW], f32, tag=f"x_dram_{wname}_{rep}",
                                   name="x_dram")
                state = {"pending": None}

                def emit_sumsq(c, sq):
                    sl_ = slice(c * NCH, (c + 1) * NCH)
                    psr = ps_row.tile([1, NCH], f32, tag="ps_row")
                    nc.tensor.matmul(psr, lhsT=ones_col, rhs=sq[:, 0, :],
                                     start=True, stop=False)
                    nc.tensor.matmul(psr, lhsT=ones_col, rhs=sq[:, 1, :],
                                     start=False, stop=True)
                    xc = rowp.tile([1, NCH], f32, tag="xrow")
                    nc.scalar.activation(xc, psr, Copy)
                    nc.gpsimd.dma_start(out=x_dram[0:1, sl_], in_=xc)

                def emit_chunk(c):
                    sl = slice(c * NCH, (c + 1) * NCH)
                    pair = in_pairs(dram_ap, sl)
                    sq = sqp.tile([P, 2, NCH], f32, tag="sq")
                    for dt_ in range(2):
                        ps = ps_big.tile([P, NCH], f32, tag="ps_big")
                        proj_mms(ps, wsb[wname], pair, dt_)
                        nc.scalar.activation(raw_sb[:, dt_, sl], ps, Ident,
                                             bias=bcol[:, dt_:dt_ + 1],
                                             scale=DPROJ)
                        nc.scalar.activation(sq[:, dt_, :], ps, Square,
                                             bias=bcol[:, dt_:dt_ + 1],
                                             scale=DPROJ)
                    if state["pending"] is not None:
                        emit_sumsq(*state["pending"])
                    state["pending"] = (c, sq)
                    if with_v:
                        for jj in range(NCH // P):
                            j = c * (NCH // P) + jj
                            nsl = slice(jj * P, (jj + 1) * P)
                            psv = ps_v.tile([P, C], f32, tag="ps_v")
                            vm = [(lc, ht, rw) for ht in range(2)
                                  for lc, rw in ((pair[0], wsb["wv"][0]),
                                                 (pair[0], wsb["wv"][1]),
                                                 (pair[1], wsb["wv"][0]))]
                            for i, (lc, ht, rw) in enumerate(vm):
                                nc.tensor.matmul(psv, lhsT=lc[:, ht, nsl],
                                                 rhs=rw[:, ht, :], start=(i == 0),
                                                 stop=(i == len(vm) - 1))
                            v_sb = vsb.tile([P, C], f32, tag="v_sb")
                            nc.vector.scalar_tensor_tensor(
                                out=v_sb, in0=psv, scalar=DPROJ,
                                in1=bv_bc, op0=alu.mult, op1=alu.add)
                            nc.sync.dma_start(out=v_dram[j * P:(j + 1) * P, :],
                                              in_=v_sb)

                def flush():
                    if state["pending"] is not None:
                        emit_sumsq(*state["pending"])
                        state["pending"] = None

                return raw_sb, x_dram, emit_chunk, flush

            def rsqrt_rows(x_dram, XW, act_scale, act_bias, tag):
                nch8 = XW // NCH
                x8f = n8x.tile([8, NCH], f32, tag="x8", name="x8f")
                nc.gpsimd.dma_start(
                    out=x8f[:nch8, :],
                    in_=x_dram[:].rearrange("o (a n) -> (o a) n", a=nch8))
                xs = x8f[:nch8, :]
                xbf = n8x.tile([8, NCH], f32, tag="x8", name="xbf")
                xb = xbf[:nch8, :]
                nc.vector.tensor_scalar(out=xb, in0=xs, scalar1=act_scale,
                                        scalar2=act_bias, op0=alu.mult,
                                        op1=alu.add)
                y0 = n8y.tile([8, NCH], f32, tag="y", name="y0")
                y = y0[:nch8, :]
                nc.scalar.activation(y, xb, Sqrt)
                nc.vector.reciprocal(out=y, in_=y)
                for _ in range(2):  # y <- y * (1.5 - 0.5 * xb * y^2)
                    a8f = n8a.tile([8, NCH], f32, tag="a8", name="a8f")
                    a8 = a8f[:nch8, :]
                    nc.vector.scalar_tensor_tensor(
                        out=a8, in0=xb, scalar=0.0, in1=y,
                        op0=alu.bypass, op1=alu.mult)
                    nc.vector.scalar_tensor_tensor(
                        out=a8, in0=a8, scalar=-0.5, in1=y,
                        op0=alu.mult, op1=alu.mult)
                    nc.vector.tensor_scalar(out=a8, in0=a8, scalar1=1.5,
                                            scalar2=None, op0=alu.add)
                    ynf = n8y.tile([8, NCH], f32, tag="y", name="ynf")
                    yn = ynf[:nch8, :]
                    nc.vector.scalar_tensor_tensor(
                        out=yn, in0=y, scalar=0.0, in1=a8,
                        op0=alu.bypass, op1=alu.mult)
                    y = yn
                r_dram = dram.tile([1, XW], f32, tag=f"r_dram_{tag}_{rep}")
                nc.gpsimd.dma_start(
                    out=r_dram[:].rearrange("o (a n) -> (o a) n", a=nch8), in_=y)
                return r_dram

            def make_prod_pair(raw_sb, r_dram, XW, col_scalar, tag, per_chunk):
                out_pairs = []
                if not per_chunk:
                    h0t = prodk_p.tile([P, 2, XW], f16, tag=tag, name="h0t")
                    h1t = prodk_p.tile([P, 2, XW], f16, tag=tag, name="h1t")
                for c in range(XW // NCH):
                    sl = slice(c * NCH, (c + 1) * NCH)
                    if per_chunk:
                        h0t = prodq_p.tile([P, 2, NCH], f16, tag=tag, name="h0c")
                        h1t = prodq_p.tile([P, 2, NCH], f16, tag=tag, name="h1c")
                        osl = slice(0, NCH)
                        out_pairs.append((h0t, h1t))
                    else:
                        osl = sl
                    rc = bcch.tile([P, NCH], f32, tag="bcch")
                    nc.sync.dma_start(out=rc,
                                      in_=r_dram[0:1, sl].to_broadcast([P, NCH]))
                    for dt_ in range(2):
                        scr = prscr.tile([P, NCH], f32, tag="prscr")
                        if col_scalar is not None:
                            nc.vector.scalar_tensor_tensor(
                                out=scr, in0=raw_sb[:, dt_, sl],
                                scalar=col_scalar[:, dt_:dt_ + 1],
                                in1=rc, op0=alu.mult, op1=alu.mult)
                        else:
                            nc.vector.scalar_tensor_tensor(
                                out=scr, in0=raw_sb[:, dt_, sl], scalar=0.0,
                                in1=rc, op0=alu.bypass, op1=alu.mult)
                        nc.vector.tensor_copy(out=h0t[:, dt_, osl], in_=scr)
                        nc.vector.scalar_tensor_tensor(
                            out=h1t[:, dt_, osl], in0=scr, scalar=0.0,
                            in1=h0t[:, dt_, osl], op0=alu.bypass,
                            op1=alu.subtract)
                return out_pairs if per_chunk else (h0t, h1t)

            # ---- k side (with v) fully, then q side ----
            k_raw, xk_dram, k_emit, k_flush = side_proj_mk(
                cbT_r, "wk", cols["bk"], NCODES, True, krawp)
            q_raw, xq_dram, q_emit, q_flush = side_proj_mk(
                hsT_r, "wq", cols["bq"], T, False, qrawp)
            for c in range(NCHUNKS):
                k_emit(c)
            k_flush()
            for c in range(NCHUNKS):
                q_emit(c)
            q_flush()
            # rk' = SK*rsqrt(sumsq/A+eps) == rsqrt((sumsq/A+eps)/SK^2)
            rk_dram = rsqrt_rows(xk_dram, NCODES, 1.0 / (A * SK * SK),
                                 EPS / (SK * SK), "k")
            # fq' = SQ*(1/sqrt(A))*rsqrt(sumsq/A+eps) == rsqrt((sumsq+A*eps)/SQ^2)
            fq_dram = rsqrt_rows(xq_dram, T, 1.0 / (SQ * SQ),
                                 A * EPS / (SQ * SQ), "q")
            ktil = make_prod_pair(k_raw, rk_dram, NCODES, cols["gg"],
                                  "prodk", False)
            qtil_c = make_prod_pair(q_raw, fq_dram, T, None, "prodq", True)

            prep_ctx.close()
            mctx = ExitStack()
            mpool = lambda name, bufs: mctx.enter_context(
                tc.tile_pool(name=f"{name}_{rep}", bufs=bufs, space="SBUF")
            )
            lgp = mpool("lgp", 3)
            ps_big2 = mctx.enter_context(
                tc.tile_pool(name=f"ps_big2_{rep}", bufs=4, space="PSUM"))
            iota_bc = mpool("iotap", 1).tile([P, NCODES], f32, tag="iota")
            nc.gpsimd.dma_start(out=iota_bc,
                                in_=iota_d[:].to_broadcast([P, NCODES]))
            scrp = mpool("scrp", 2)
            smal = mpool("smal", 4)
            zqp = mpool("zqp", 3)
            idx_all = mpool("idxall", 1).tile([P, TT], i32, tag="idxall")

            for tt in range(TT):
                tsl = slice(tt * P, (tt + 1) * P)
                lg = lgp.tile([P, NCODES], f32, tag="lg")
                qtil = qtil_c[tt // 4]
                qsl = slice((tt % 4) * P, (tt % 4 + 1) * P)
                for c in range(NCHUNKS):
                    sl = slice(c * NCH, (c + 1) * NCH)
                    ps = (ps_big if c % 2 == 0 else ps_big2).tile(
                        [P, NCH], f32, tag="ps_big")
                    bmms = [(lq, dt_, rk_) for dt_ in range(2)
                            for lq, rk_ in ((qtil[0], ktil[0]),
                                            (qtil[0], ktil[1]),
                                            (qtil[1], ktil[0]))]
                    for i, (lq, dt_, rk_) in enumerate(bmms):
                        nc.tensor.matmul(ps, lhsT=lq[:, dt_, qsl],
                                         rhs=rk_[:, dt_, sl],
                                         start=(i == 0),
                                         stop=(i == len(bmms) - 1))
                    nc.scalar.activation(lg[:, sl], ps, Copy, scale=DBIG)
                nc.sync.dma_start(out=logits_o[tsl, :NCODES // 2],
                                  in_=lg[:, :NCODES // 2])
                nc.sync.dma_start(out=logits_o[tsl, NCODES // 2:],
                                  in_=lg[:, NCODES // 2:])

                # argmax: pairwise max tree, then (lg == m) * iota with
                # free-dim sum accumulation (split DVE/GPSIMD by tile parity)
                mx4 = scrp.tile([P, 4 * NCH], f32, tag="mx4")
                for i in range(4):
                    nc.vector.scalar_tensor_tensor(
                        out=mx4[:, i * NCH:(i + 1) * NCH],
                        in0=lg[:, (2 * i) * NCH:(2 * i + 1) * NCH], scalar=0.0,
                        in1=lg[:, (2 * i + 1) * NCH:(2 * i + 2) * NCH],
                        op0=alu.bypass, op1=alu.max)
                mx2 = scrp.tile([P, 2 * NCH], f32, tag="mx2")
                for i in range(2):
                    nc.vector.scalar_tensor_tensor(
                        out=mx2[:, i * NCH:(i + 1) * NCH],
                        in0=mx4[:, (2 * i) * NCH:(2 * i + 1) * NCH], scalar=0.0,
                        in1=mx4[:, (2 * i + 1) * NCH:(2 * i + 2) * NCH],
                        op0=alu.bypass, op1=alu.max)
                mx1 = scrp.tile([P, NCH], f32, tag="mx1")
                nc.vector.scalar_tensor_tensor(
                    out=mx1, in0=mx2[:, :NCH], scalar=0.0, in1=mx2[:, NCH:],
                    op0=alu.bypass, op1=alu.max)
                m = smal.tile([P, 1], f32, tag="m")
                nc.vector.reduce_max(out=m, in_=mx1, axis=AX)

                eq_eng = nc.vector
                acc = smal.tile([P, NCHUNKS], f32, tag="acc")
                for c in range(NCHUNKS):
                    sl = slice(c * NCH, (c + 1) * NCH)
                    scr = scrp.tile([P, NCH], f32, tag="scr")
                    eq_eng.scalar_tensor_tensor(
                        out=scr, in0=lg[:, sl], scalar=m[:, 0:1],
                        in1=iota_bc[:, sl], op0=alu.is_equal, op1=alu.mult,
                        accum_out=acc[:, c:c + 1])
                idxf = smal.tile([P, 1], f32, tag="idxf")
                nc.vector.reduce_sum(out=idxf, in_=acc, axis=AX)
                nc.vector.tensor_copy(out=idx_all[:, tt:tt + 1], in_=idxf)

                zq = zqp.tile([P, C], f32, tag="zq")
                nc.gpsimd.indirect_dma_start(
                    out=zq[:, :], out_offset=None, in_=v_dram[:, :],
                    in_offset=bass.IndirectOffsetOnAxis(
                        ap=idx_all[:, tt:tt + 1], axis=0))
                nc.sync.dma_start(out=zq_o[tsl, :], in_=zq)
                nc.sync.dma_start(out=zq2_o[tsl, :], in_=zq)

            nc.gpsimd.dma_start(
                out=idx_o[:].rearrange("(a p) o -> p (a o)", p=P), in_=idx_all)
            mctx.close()

    if not nc.is_finalized():
        nc.finalize()
    return nc


_NC_CACHE = {}


def _get_nc(reps=1):
    if reps not in _NC_CACHE:
        _NC_CACHE[reps] = _build_module(reps)
    return _NC_CACHE[reps]


def kernel(hidden_states, codebook_hidden_states, wq, bq, wk, bk, wv, bv, gq, gk):
    from concourse.bass_utils import run_bass_kernel_spmd

    hs = np.ascontiguousarray(np.asarray(hidden_states, dtype=np.float32))
    cb = np.ascontiguousarray(np.asarray(codebook_hidden_states, dtype=np.float32))
    wq = np.ascontiguousarray(np.asarray(wq, dtype=np.float32))
    wk = np.ascontiguousarray(np.asarray(wk, dtype=np.float32))
    wv = np.ascontiguousarray(np.asarray(wv, dtype=np.float32))
    bq = np.asarray(bq, dtype=np.float32).reshape(1, A)
    bk = np.asarray(bk, dtype=np.float32).reshape(1, A)
    bv = np.asarray(bv, dtype=np.float32).reshape(1, C)
    gq = np.asarray(gq, dtype=np.float32).reshape(1, A)
    gk = np.asarray(gk, dtype=np.float32).reshape(1, A)

    hsT = np.ascontiguousarray(hs.T)          # [H, 32768]
    cbT = np.ascontiguousarray(cb.T)          # [H, 4096]
    iota = np.arange(NCODES, dtype=np.float32).reshape(1, NCODES)

    nc = _get_nc()
    in_maps = []
    for core in range(N_CORES):
        sl = slice(core * T, (core + 1) * T)
        in_maps.append({
            "hsT": np.ascontiguousarray(hsT[:, sl]),
            "cbT": cbT, "wq": wq, "wk": wk, "wv": wv,
            "bq": bq, "bk": bk, "bv": bv, "gq": gq, "gk": gk,
            "iota": iota,
        })

    res = run_bass_kernel_spmd(nc, in_maps, list(range(N_CORES)))
    kernel.last_results = res

    logits = np.concatenate([res.results[i]["logits"] for i in range(N_CORES)],
                            axis=0)
    idx = np.concatenate([res.results[i]["idx"] for i in range(N_CORES)], axis=0)
    zq = np.concatenate([res.results[i]["zq"] for i in range(N_CORES)], axis=0)
    zq2 = np.concatenate([res.results[i]["zq2"] for i in range(N_CORES)], axis=0)
    return (logits, idx.astype(np.int32), zq, zq2)
